# revision 11
# baseline (speedup 1.0000x reference)
"""Distributed Bass kernel for sparse cluster attention on 8 TRN2 NeuronCores.

v2: pure head-parallel, collective-minimal design.
  - Host replicates x^T (bf16) + keyframe x hi/lo to every core and
    pre-slices W_qkv columns per core's 2 heads.  No weight AllGather,
    no qkv AllToAll.
  - Phase B: fp32-accurate keyframe q.k scores for the core's 2 heads
    over all 2048 keyframe tokens (hi/lo bf16, 3 matmul passes),
    per-head reduce + max -> [2048] -> tiny AllReduce(max) (hidden
    behind phase A).
  - Phase A: qkv for 2 heads over all 16384 tokens from replicated x:
    qT [128ch, N] direct; k/v produced channel-major then PE-transposed
    into token-major kv_sb [tok, k|v] for the gathers.
  - Phase 3: on-device top-153 per cluster via rank comparison
    (identical on every core).
  - Phase 3b: fused one-hot gather matmuls: kv rows [sel, k|v] in one
    256-wide matmul per token-chunk; k half transposed into kg.
  - Phase 4: flash-style attention per consumer cluster, explicitly
    software-pipelined: logits(g) [PE] -> exp(g) [ACT] -> AV(g-1) [PE].
  - Phase 5: output halves AllToAll'd while the other half computes;
    head-parallel row blocks -> token-parallel proj -> out f16.
"""

import numpy as np
import ml_dtypes

import os
import concourse.bass as bass
import concourse.bacc as bacc
import concourse.mybir as mybir
import concourse.tile as tile
from concourse.bass_utils import run_bass_kernel_spmd

BF16 = mybir.dt.bfloat16
F32 = mybir.dt.float32
F16 = mybir.dt.float16
I32 = mybir.dt.int32
AF = mybir.ActivationFunctionType
OP = mybir.AluOpType

# problem constants
H, D, C = 16, 64, 1024
S, P = 32, 512
K, FC = 4, 8
N = S * P                      # 16384 tokens
TK = 153                       # top-k patches per cluster
NSUB = 5                       # subsampled frames
NCORES = 8
HC = H // NCORES               # heads per core = 2
QCH = HC * D                   # q channels per core = 128
TOKS = N // NCORES             # tokens per core out slice = 2048
KFT = K * P                    # keyframe tokens = 2048
SCALE = float(D) ** -0.5
FULL = FC * TK                 # packed kv rows per full src block = 1224
PRE5 = NSUB * TK               # packed kv rows per 5-frame prefix = 765
KGW = 1280                     # kg tile width (>= FULL, mult of 128)
NT = N // 512                  # 32 token tiles of 512

_CACHE: dict = {}


def _chunks_for(ci):
    """(src, chunk, rows) list for consumer cluster ci over packed kv."""
    out = []
    for src in range(K):
        valid = FULL if src in (0, ci) else PRE5
        nch = (valid + 127) // 128
        for c in range(nch):
            out.append((src, c, min(128, valid - c * 128)))
    return out


def _win_frames(c):
    """Frames whose packed rows [f*153, (f+1)*153) intersect window
    [128c, 128(c+1))."""
    lo, hi = 128 * c, 128 * (c + 1)
    return [f for f in range(FC) if f * TK < hi and (f + 1) * TK > lo]


def build_nc(clusters, keyframes):
    STUB = os.environ.get("KSTUB", "0") == "1"
    KCUT = int(os.environ.get("KCUT", "9"))
    KREP = int(os.environ.get("KREP", "1"))
    nc = bacc.Bacc(None, target_bir_lowering=False, debug=False)

    # ---- kernel I/O ----
    xT = nc.dram_tensor("xT", [C, N], BF16, kind="ExternalInput")
    xkfT_h = nc.dram_tensor("xkfT_h", [C, KFT], BF16, kind="ExternalInput")
    xkfT_l = nc.dram_tensor("xkfT_l", [C, KFT], BF16, kind="ExternalInput")
    wq = nc.dram_tensor("wq", [C, QCH], BF16, kind="ExternalInput")
    wkv = nc.dram_tensor("wkv", [C, 2 * QCH], BF16, kind="ExternalInput")
    wqk_h = nc.dram_tensor("wqk_h", [C, 2 * QCH], BF16, kind="ExternalInput")
    wqk_l = nc.dram_tensor("wqk_l", [C, 2 * QCH], BF16, kind="ExternalInput")
    wpj = nc.dram_tensor("wpj", [C, C], BF16, kind="ExternalInput")
    bqs = nc.dram_tensor("bqs", [3 * QCH], F32, kind="ExternalInput")
    bpj = nc.dram_tensor("bpj", [C], F32, kind="ExternalInput")
    out_ext = nc.dram_tensor("out", [TOKS, C], F16, kind="ExternalOutput")
    KDBG = os.environ.get("KDBG", "0") == "1"
    if KDBG:
        dbg_qT = nc.dram_tensor("dbg_qT", [128, N], BF16, kind="ExternalOutput")
        dbg_kv = nc.dram_tensor("dbg_kv", [128, N // 128, 2 * QCH], BF16, kind="ExternalOutput")
        dbg_sc = nc.dram_tensor("dbg_sc", [KFT], F32, kind="ExternalOutput")
        dbg_kg = nc.dram_tensor("dbg_kg", [128, K * KGW], BF16, kind="ExternalOutput")
        dbg_va = nc.dram_tensor("dbg_va", [128, K * 10 * 130], BF16, kind="ExternalOutput")

    # ---- internal DRAM ----
    sc_in = nc.dram_tensor("sc_in", [KFT], F32)
    sc_out = nc.dram_tensor("sc_out", [KFT], F32, addr_space="Shared")
    ag_in = [nc.dram_tensor(f"ag_in{i}", [NCORES, QCH, TOKS // 2], BF16) for i in range(2)]
    ag_out = [nc.dram_tensor(f"ag_out{i}", [NCORES, QCH, TOKS // 2], BF16)
              for i in range(2)]

    def coll(kind, op, ins, outs):
        nc.gpsimd.collective_compute(
            kind, op, replica_groups=[list(range(NCORES))], ins=ins, outs=outs)

    if STUB:
        with tile.TileContext(nc) as tc:
            with tc.tile_pool(name="sp", bufs=2) as sp:
                t = sp.tile([128, 512], BF16)
                nc.sync.dma_start(t[:], xT.ap()[0:128, 0:512])
                t2 = sp.tile([128, 512], F16)
                nc.vector.tensor_copy(t2[:], t[:])
                nc.sync.dma_start(out_ext.ap()[0:128, 0:512], t2[:])
        nc.finalize()
        return nc

    # per-cluster frame lists and half split (A2A halves by token range)
    frames = [[int(clusters[ci][j]) for j in range(FC)] for ci in range(K)]
    half_qt = []
    for half in range(2):
        hq = {}
        for ci in range(K):
            hq[ci] = [j for j in range(FC)
                      if ((frames[ci][j] * P) % TOKS) // (TOKS // 2) == half]
        half_qt.append(hq)

    with tile.TileContext(nc) as tc:
        with (
            tc.tile_pool(name="persist", bufs=1) as pp,
            tc.tile_pool(name="work", bufs=2) as wp,
            tc.tile_pool(name="xp", bufs=2) as xp,
            tc.tile_pool(name="kp", bufs=3) as kp,
            tc.tile_pool(name="expw", bufs=3) as ep,
            tc.tile_pool(name="pslg", bufs=2, space="PSUM") as psL,
            tc.tile_pool(name="psav", bufs=2, space="PSUM") as psV,
            tc.tile_pool(name="psmed", bufs=2, space="PSUM") as psM,
        ):
            # ================= persistent SBUF + weights =================
            qT = pp.tile([128, N], BF16, tag="qT")                  # 32KB/p
            kv_sb = pp.tile([128, N // 128, 2 * QCH], BF16, tag="kvsb")  # 64KB/p
            kg = pp.tile([128, K, KGW], BF16, tag="kg")             # 10KB/p
            vaug = pp.tile([128, K, 10, 130], BF16, tag="vaug")     # 10.2KB/p

            wq_sb = pp.tile([128, 8, QCH], BF16, tag="wq")
            wkv_sb = pp.tile([128, 8, 2 * QCH], BF16, tag="wkv")
            wqkh_sb = pp.tile([128, 8, 2 * QCH], BF16, tag="wqkh")
            wqkl_sb = pp.tile([128, 8, 2 * QCH], BF16, tag="wqkl")
            for t_, src_ in ((wq_sb, wq), (wkv_sb, wkv), (wqkh_sb, wqk_h),
                             (wqkl_sb, wqk_l)):
                nc.sync.dma_start(t_[:], src_.ap().rearrange("(a p) c -> p a c", p=128))

            # biases: bqs = [q 128 | k 128 | v 128] f32 -> bf16 row
            bq_f = wp.tile([1, 3 * QCH], F32, tag="srow", bufs=1, name="bq_f")
            nc.sync.dma_start(bq_f[:], bqs.ap().rearrange("(a c) -> a c", a=1))
            bq_b = pp.tile([1, 3 * QCH], BF16, tag="bqb")
            nc.vector.tensor_copy(bq_b[:], bq_f[:])
            bpj_f = wp.tile([1, C], F32, tag="srow", bufs=1, name="bpj_f")
            nc.sync.dma_start(bpj_f[:], bpj.ap().rearrange("(a c) -> a c", a=1))
            bpj_b = pp.tile([1, C], BF16, tag="bpjb")
            nc.vector.tensor_copy(bpj_b[:], bpj_f[:])

            ones_b = pp.tile([1, 512], BF16, tag="onesb")
            nc.vector.memset(ones_b[:], 1.0)
            onesf = pp.tile([1, 128], F32, tag="onesf")
            nc.vector.memset(onesf[:], 1.0)

            # identity matrices + iotas
            it_row = wp.tile([128, 128], I32, tag="i32w", bufs=1)
            nc.gpsimd.iota(it_row[:], pattern=[[1, 128]], base=0, channel_multiplier=0)
            it_col = wp.tile([128, 1], I32, tag="i32c", bufs=1)
            nc.gpsimd.iota(it_col[:], pattern=[[1, 1]], base=0, channel_multiplier=1)
            rowf = wp.tile([128, 128], F32, tag="rowf", bufs=1)
            nc.vector.tensor_copy(rowf[:], it_row[:])
            colf = wp.tile([128, 1], F32, tag="colf", bufs=1)
            nc.vector.tensor_copy(colf[:], it_col[:])
            ident = pp.tile([128, 128], BF16, tag="ident")
            nc.vector.tensor_scalar(ident[:], rowf[:], colf[:], None, OP.is_equal)

            iota_tc = wp.tile([128, 128], I32, tag="i32w", bufs=1, name="iota_tc")
            nc.gpsimd.iota(iota_tc[:], pattern=[[128, 128]], base=0, channel_multiplier=1)
            iota_tcf = pp.tile([128, 128], F32, tag="iotcf")
            nc.vector.tensor_copy(iota_tcf[:], iota_tc[:])
            iota160 = wp.tile([128, 160], I32, tag="i32w", bufs=1, name="iota160")
            nc.gpsimd.iota(iota160[:], pattern=[[1, 160]], base=0, channel_multiplier=0)
            iota160f = pp.tile([128, 160], F32, tag="io160f")
            nc.vector.tensor_copy(iota160f[:], iota160[:])
            iota_pv = wp.tile([128, 4], I32, tag="i32c", bufs=1, name="iota_pv")
            nc.gpsimd.iota(iota_pv[:], pattern=[[128, 4]], base=0, channel_multiplier=1)
            iota_pvf = pp.tile([128, 4], F32, tag="iopvf")
            nc.vector.tensor_copy(iota_pvf[:], iota_pv[:])

            wpj_sb = pp.tile([128, 8, C], BF16, tag="wpj")
            nc.sync.dma_start(wpj_sb[:], wpj.ap().rearrange("(a p) c -> p a c", p=128))

          # ================= repeated body (KREP for timing) =================
          # (indentation kept flat: rep loop wraps phases B..5)
            for rep in range(KREP):
              # ================= phase B: keyframe scores =================
              if KCUT >= 1:
                for t2 in range(KFT // 128):
                    xkh = kp.tile([128, 8, 128], BF16, tag="xkf", name=f"xkh{rep}_{t2}")
                    xkl = kp.tile([128, 8, 128], BF16, tag="xkf", name=f"xkl{rep}_{t2}")
                    nc.sync.dma_start(
                        xkh[:], xkfT_h.ap()[:, t2 * 128:(t2 + 1) * 128]
                        .rearrange("(a p) k -> p a k", p=128))
                    nc.sync.dma_start(
                        xkl[:], xkfT_l.ap()[:, t2 * 128:(t2 + 1) * 128]
                        .rearrange("(a p) k -> p a k", p=128))
                    psb = psL.tile([128, 2 * QCH], F32, tag="lg", name=f"psb{rep}_{t2}")
                    first = True
                    for w_, x_ in ((wqkh_sb, xkh), (wqkh_sb, xkl), (wqkl_sb, xkh)):
                        for cc in range(8):
                            nc.tensor.matmul(psb[:], x_[:, cc, :], w_[:, cc, :],
                                             start=first, stop=False)
                            first = False
                    nc.tensor.matmul(psb[:], ones_b[:, 0:128], bq_b[:, 0:2 * QCH],
                                     start=False, stop=True)
                    qk_s = wp.tile([128, 2 * QCH], F32, tag="qks", bufs=1)
                    nc.vector.tensor_copy(qk_s[:], psb[:])
                    qs = wp.tile([128, QCH], F32, tag="qs", bufs=1)
                    nc.vector.tensor_tensor(qs[:], qk_s[:, 0:QCH], qk_s[:, QCH:2 * QCH],
                                            OP.mult)
                    hs = wp.tile([128, HC], F32, tag="hs", bufs=2)
                    for h in range(HC):
                        nc.vector.reduce_sum(hs[:, h:h + 1], qs[:, h * D:(h + 1) * D],
                                             axis=mybir.AxisListType.X)
                    smax_c = wp.tile([128, 1], F32, tag="smaxc", bufs=2)
                    nc.vector.reduce_max(smax_c[:], hs[:], axis=mybir.AxisListType.X)
                    nc.sync.dma_start(
                        sc_in.ap()[t2 * 128:(t2 + 1) * 128].rearrange("(p a) -> p a", a=1),
                        smax_c[:])
                coll("AllReduce", OP.max, [sc_in.ap().opt()], [sc_out.ap().opt()])

              # ================= phase A: qkv all tokens, 2 heads ===========
              if KCUT >= 2:
                for tt in range(NT):
                    xt = xp.tile([128, 8, 512], BF16, tag="xt", name=f"xt{rep}_{tt}")
                    nc.sync.dma_start(
                        xt[:], xT.ap()[:, tt * 512:(tt + 1) * 512]
                        .rearrange("(a p) t -> p a t", p=128))
                    psq = psL.tile([128, 512], F32, tag="lg", name=f"psq{rep}_{tt}")
                    for cc in range(8):
                        nc.tensor.matmul(psq[:], wq_sb[:, cc, :], xt[:, cc, :],
                                         start=(cc == 0), stop=False)
                    nc.tensor.matmul(psq[:], bq_b[:, 0:QCH], ones_b[:],
                                     start=False, stop=True)
                    nc.scalar.copy(qT[:, tt * 512:(tt + 1) * 512], psq[:])
                    for kvc in range(2):
                        pskv = psL.tile([128, 512], F32, tag="lg",
                                        name=f"pskv{rep}_{tt}_{kvc}")
                        for cc in range(8):
                            nc.tensor.matmul(pskv[:], wkv_sb[:, cc, kvc * 128:(kvc + 1) * 128],
                                             xt[:, cc, :], start=(cc == 0), stop=False)
                        nc.tensor.matmul(pskv[:], bq_b[:, (1 + kvc) * 128:(2 + kvc) * 128],
                                         ones_b[:], start=False, stop=True)
                        kvT = ep.tile([128, 512], BF16, tag="kvT", bufs=2)
                        nc.vector.tensor_copy(kvT[:], pskv[:])
                        pst = psM.tile([128, 512], BF16, tag="med",
                                       name=f"pst{rep}_{tt}_{kvc}")
                        for s4 in range(4):
                            nc.tensor.transpose(pst[:, s4 * 128:(s4 + 1) * 128],
                                                kvT[:, s4 * 128:(s4 + 1) * 128], ident[:])
                        nc.vector.tensor_copy(
                            kv_sb[:, tt * 4:(tt + 1) * 4, kvc * 128:(kvc + 1) * 128],
                            pst[:].rearrange("p (a k) -> p a k", k=128))

              if KDBG and KCUT >= 2:
                nc.sync.dma_start(dbg_qT.ap(), qT[:])
                nc.sync.dma_start(dbg_kv.ap(), kv_sb[:])
              if KDBG and KCUT >= 1:
                scdbg = wp.tile([128, 16], F32, tag="qks", bufs=1, name="scdbg")
                nc.sync.dma_start(scdbg[:], sc_out.ap().rearrange("(t p) -> p t", p=128))
                nc.sync.dma_start(dbg_sc.ap().rearrange("(t p) -> p t", p=128), scdbg[:])
              # ================= phase 3: top-k -> packed patch-id rows =====
              psel_rows = {}
              if KCUT >= 3:
                for cl in range(K):
                    s_row = wp.tile([1, P], F32, tag="srow", bufs=1)
                    nc.sync.dma_start(
                        s_row[:], sc_out.ap()[cl * P:(cl + 1) * P]
                        .rearrange("(a c) -> a c", a=1))
                    s_colT = wp.tile([128, 4], F32, tag="scolT", bufs=1)
                    nc.sync.dma_start(
                        s_colT[:], sc_out.ap()[cl * P:(cl + 1) * P]
                        .rearrange("(a p) -> p a", p=128))
                    ps_bc = psM.tile([128, P], F32, tag="med", name=f"psbc{rep}_{cl}")
                    nc.tensor.matmul(ps_bc[:], onesf[:], s_row[:], start=True, stop=True)
                    s_bc = wp.tile([128, P], F32, tag="sbc", bufs=1)
                    nc.vector.tensor_copy(s_bc[:], ps_bc[:])
                    ps_row = psM.tile([1, 160], F32, tag="med", name=f"psrow{rep}_{cl}")
                    for pc in range(4):
                        gt = wp.tile([128, P], BF16, tag="gtm", bufs=1)
                        nc.vector.tensor_scalar(gt[:], s_bc[:], s_colT[:, pc:pc + 1],
                                                None, OP.is_gt)
                        rank = wp.tile([128, 1], F32, tag="rank", bufs=2)
                        nc.vector.reduce_sum(rank[:], gt[:], axis=mybir.AxisListType.X)
                        eqr = wp.tile([128, 160], F32, tag="eqr", bufs=1)
                        nc.vector.tensor_scalar(eqr[:], iota160f[:], rank[:],
                                                None, OP.is_equal)
                        nc.tensor.matmul(ps_row[:], iota_pvf[:, pc:pc + 1], eqr[:],
                                         start=(pc == 0), stop=(pc == 3))
                    psel_row = pp.tile([1, 160], F32, tag=f"pselr{cl}",
                                       name=f"pselr{rep}_{cl}")
                    nc.vector.tensor_copy(psel_row[:], ps_row[:])
                    psel_rows[cl] = psel_row

                # ---- phase 3b: fused one-hot gathers ----
                for src in range(K):
                    psB2 = wp.tile([128, KGW], F32, tag="psB2", bufs=1)
                    nc.vector.memset(psB2[:, FULL:KGW], -1.0)
                    for f8 in range(FC):
                        fr = frames[src][f8]
                        ps_b = psM.tile([128, 512], F32, tag="med",
                                        name=f"psb2_{rep}_{src}_{f8}")
                        nc.tensor.matmul(ps_b[:, 0:160], onesf[:], psel_rows[src][:],
                                         start=True, stop=True)
                        nc.vector.tensor_scalar(psB2[:, f8 * TK:(f8 + 1) * TK],
                                                ps_b[:, 0:TK], float(fr * P), None, OP.add)
                    for c in range(10):
                        tcs = []
                        for f8 in _win_frames(c):
                            fr = frames[src][f8]
                            tcs.extend(fr * 4 + i for i in range(4))
                        psg = psM.tile([128, 2 * QCH], F32, tag="med",
                                       name=f"psg{rep}_{src}_{c}")
                        for ti, tc_ in enumerate(tcs):
                            ohW = wp.tile([128, 128], BF16, tag="ohW", bufs=2,
                                          name=f"ohW{rep}_{src}_{c}_{ti}")
                            nc.vector.tensor_scalar(ohW[:], psB2[:, c * 128:(c + 1) * 128],
                                                    iota_tcf[:, tc_:tc_ + 1], None,
                                                    OP.is_equal)
                            nc.tensor.matmul(psg[:], ohW[:], kv_sb[:, tc_, :],
                                             start=(ti == 0), stop=(ti == len(tcs) - 1))
                        stage = wp.tile([128, 2 * QCH], BF16, tag="stg", bufs=2)
                        nc.vector.tensor_copy(stage[:], psg[:])
                        nc.vector.tensor_copy(vaug[:, src, c, 0:64], stage[:, 128:192])
                        nc.vector.tensor_copy(vaug[:, src, c, 65:129], stage[:, 192:256])
                        pst2 = psM.tile([128, 128], BF16, tag="med",
                                        name=f"pst2_{rep}_{src}_{c}")
                        nc.tensor.transpose(pst2[:], stage[:, 0:128], ident[:])
                        nc.vector.tensor_copy(kg[:, src, c * 128:(c + 1) * 128], pst2[:])
                nc.vector.memset(vaug[:, :, :, 64:65], 1.0)
                nc.vector.memset(vaug[:, :, :, 129:130], 1.0)

              if KDBG and KCUT >= 3:
                nc.sync.dma_start(dbg_kg.ap(), kg[:].rearrange("p a b -> p (a b)"))
                nc.sync.dma_start(dbg_va.ap(), vaug[:].rearrange("p a b c -> p (a b c)"))
              # ================= phase 4: attention, half-pipelined =========
              if KCUT >= 4:
                for half in range(2):
                    for ci in range(K):
                        chunks = _chunks_for(ci)
                        nchk = len(chunks)
                        for qt in half_qt[half][ci]:
                            f_q = frames[ci][qt]
                            qsl = slice(f_q * P, (f_q + 1) * P)
                            ps_av = [psV.tile([65, 512], F32, tag="av",
                                              name=f"psav{rep}_{ci}_{qt}_{i}")
                                     for i in range(2)]
                            pend = None  # (g, ew, rows, src, c) awaiting AV
                            for g, (src, c, rows) in enumerate(chunks):
                                ps_lg = psL.tile([128, 1024], F32, tag="lg", bufs=2,
                                                 name=f"pslg{rep}_{ci}_{qt}_{g}")
                                for h in range(2):
                                    nc.tensor.matmul(
                                        ps_lg[:, h * 512:(h + 1) * 512],
                                        kg[h * 64:(h + 1) * 64, src, c * 128:(c + 1) * 128],
                                        qT[h * 64:(h + 1) * 64, qsl],
                                        start=True, stop=True,
                                        tile_position=(h * 64, 0))
                                ew = ep.tile([128, 1024], BF16, tag="ew", bufs=2)
                                nc.scalar.activation(ew[:], ps_lg[:], AF.Exp, scale=SCALE)
                                if pend is not None:
                                    pg, pew, prows, psrc, pc_ = pend
                                    for h in range(2):
                                        nc.tensor.matmul(
                                            ps_av[h][:],
                                            vaug[0:prows, psrc, pc_, h * 65:(h + 1) * 65],
                                            pew[0:prows, h * 512:(h + 1) * 512],
                                            start=(pg == 0), stop=False)
                                pend = (g, ew, rows, src, c)
                            pg, pew, prows, psrc, pc_ = pend
                            for h in range(2):
                                nc.tensor.matmul(
                                    ps_av[h][:],
                                    vaug[0:prows, psrc, pc_, h * 65:(h + 1) * 65],
                                    pew[0:prows, h * 512:(h + 1) * 512],
                                    start=(pg == 0), stop=True)
                            # normalize -> A2A staging
                            otile = ep.tile([128, 512], BF16, tag="ot", bufs=2)
                            for h in range(2):
                                rec = wp.tile([1, 512], F32, tag="rec", bufs=1)
                                nc.vector.reciprocal(rec[:], ps_av[h][64:65, :])
                                ps_bc2 = psM.tile([64, 512], F32, tag="med",
                                                  name=f"psbc2_{rep}_{ci}_{qt}_{h}")
                                nc.tensor.matmul(ps_bc2[:], onesf[:, 0:64], rec[:],
                                                 start=True, stop=True)
                                bc_sb = wp.tile([64, 512], F32, tag="bcsb", bufs=1)
                                nc.vector.tensor_copy(bc_sb[:], ps_bc2[:])
                                nc.vector.tensor_tensor(
                                    otile[h * 64:(h + 1) * 64, :],
                                    ps_av[h][0:64, :], bc_sb[:], OP.mult)
                            jcore = (f_q * P) // TOKS
                            toff = (f_q * P) % TOKS % (TOKS // 2)
                            nc.sync.dma_start(
                                ag_in[half].ap()[jcore, :, toff:toff + 512], otile[:])
                    coll("AllToAll", OP.bypass,
                         [ag_in[half].ap().opt()], [ag_out[half].ap().opt()])

              # ================= phase 5: proj, half-pipelined ==============
              if KCUT >= 5:
                for half in range(2):
                    for quarter in range(2):
                        atk = xp.tile([128, 8, 512], BF16, tag="xt",
                                      name=f"atk{rep}_{half}_{quarter}")
                        nc.sync.dma_start(
                            atk[:],
                            ag_out[half].ap()[:, :, quarter * 512:(quarter + 1) * 512]
                            .rearrange("j p t -> p j t"))
                        for mt in range(4):
                            gmt = half * 8 + quarter * 4 + mt
                            for ntile in range(2):
                                nsl = slice(ntile * 512, (ntile + 1) * 512)
                                ps = psM.tile([128, 512], F32, tag="med",
                                              name=f"pspj{rep}_{gmt}_{ntile}")
                                for cc in range(8):
                                    nc.tensor.matmul(ps[:], atk[:, cc, mt * 128:(mt + 1) * 128],
                                                     wpj_sb[:, cc, nsl],
                                                     start=(cc == 0), stop=False)
                                nc.tensor.matmul(ps[:], ones_b[:, 0:128], bpj_b[:, nsl],
                                                 start=False, stop=True)
                                ot = wp.tile([128, 512], F16, tag="otile", bufs=2)
                                nc.scalar.copy(ot[:], ps[:])
                                nc.sync.dma_start(
                                    out_ext.ap()[gmt * 128:(gmt + 1) * 128, nsl], ot[:])

    nc.finalize()
    return nc


def _host_prep(x, W_qkv, b_qkv, W_proj, b_proj, clusters, keyframes):
    bf = ml_dtypes.bfloat16
    x2 = np.ascontiguousarray(x.reshape(N, C))
    xT = np.ascontiguousarray(x2.T.astype(bf))                     # [C, N]
    kf_tok = np.concatenate([np.arange(P, dtype=np.int64) + int(f) * P
                             for f in keyframes])
    xkf = x2[kf_tok]                                               # [2048, C] f32
    xkf_h = xkf.astype(bf)
    xkf_l = (xkf - xkf_h.astype(np.float32)).astype(bf)
    xkfT_h = np.ascontiguousarray(xkf_h.T)
    xkfT_l = np.ascontiguousarray(xkf_l.T)
    wpj_b = np.ascontiguousarray(W_proj.astype(bf))
    bpj_f = np.ascontiguousarray(b_proj.astype(np.float32))

    in_maps = []
    for core in range(NCORES):
        h0 = core * HC
        qcols = np.arange(h0 * D, h0 * D + QCH)
        wq_s = W_qkv[:, qcols]
        wk_s = W_qkv[:, C + qcols]
        wv_s = W_qkv[:, 2 * C + qcols]
        wqk = np.concatenate([wq_s, wk_s], axis=1)                 # [C, 256] f32
        wqk_hi = wqk.astype(bf)
        wqk_lo = (wqk - wqk_hi.astype(np.float32)).astype(bf)
        bqs_s = np.concatenate([b_qkv[qcols], b_qkv[C + qcols],
                                b_qkv[2 * C + qcols]]).astype(np.float32)
        in_maps.append({
            "xT": xT,
            "xkfT_h": xkfT_h,
            "xkfT_l": xkfT_l,
            "wq": np.ascontiguousarray(wq_s.astype(bf)),
            "wkv": np.ascontiguousarray(
                np.concatenate([wk_s, wv_s], axis=1).astype(bf)),
            "wqk_h": np.ascontiguousarray(wqk_hi),
            "wqk_l": np.ascontiguousarray(wqk_lo),
            "wpj": wpj_b,
            "bqs": np.ascontiguousarray(bqs_s),
            "bpj": bpj_f,
        })
    return in_maps


def kernel(x, W_qkv, b_qkv, W_proj, b_proj, clusters, keyframes, **run_kwargs):
    x = np.asarray(x, dtype=np.float32)
    W_qkv = np.asarray(W_qkv, dtype=np.float32)
    b_qkv = np.asarray(b_qkv, dtype=np.float32)
    W_proj = np.asarray(W_proj, dtype=np.float32)
    b_proj = np.asarray(b_proj, dtype=np.float32)
    clusters = np.asarray(clusters, dtype=np.int32)
    keyframes = np.asarray(keyframes, dtype=np.int32)

    key = (clusters.tobytes(), keyframes.tobytes(), os.environ.get("KSTUB"),
           os.environ.get("KCUT"), os.environ.get("KREP"), os.environ.get("KDBG"))
    if _CACHE.get("key") != key:
        _CACHE["nc"] = build_nc(clusters, keyframes)
        _CACHE["key"] = key
    nc = _CACHE["nc"]

    in_maps = _host_prep(x, W_qkv, b_qkv, W_proj, b_proj, clusters, keyframes)
    res = run_bass_kernel_spmd(nc, in_maps, core_ids=list(range(NCORES)), **run_kwargs)
    _CACHE["last_result"] = res
    outs = res.results
    full = np.concatenate([np.asarray(outs[c]["out"], dtype=np.float32)
                           for c in range(NCORES)], axis=0)
    return full.reshape(1, N, C)


def bench(x, W_qkv, b_qkv, W_proj, b_proj, clusters, keyframes, iters=10, reps=5):
    """Steady-state on-device timing: times the best of `reps` calls."""
    import time
    import jax
    from jax.sharding import Mesh, PartitionSpec
    from jax.experimental.shard_map import shard_map
    from concourse import bass2jax
    from concourse.bass2jax import _bass_exec_p
    import concourse.mybir as _mb

    clusters = np.asarray(clusters, dtype=np.int32)
    keyframes = np.asarray(keyframes, dtype=np.int32)
    key = (clusters.tobytes(), keyframes.tobytes(), os.environ.get("KSTUB"),
           os.environ.get("KCUT"), os.environ.get("KREP"), os.environ.get("KDBG"))
    if _CACHE.get("key") != key:
        _CACHE["nc"] = build_nc(clusters, keyframes)
        _CACHE["key"] = key
    nc = _CACHE["nc"]
    bass2jax.install_neuronx_cc_hook()

    in_maps = _host_prep(np.asarray(x, np.float32), np.asarray(W_qkv, np.float32),
                         np.asarray(b_qkv, np.float32), np.asarray(W_proj, np.float32),
                         np.asarray(b_proj, np.float32), clusters, keyframes)

    in_names, out_names, out_avals, zero_outs = [], [], [], []
    partition_name = nc.partition_id_tensor.name if nc.partition_id_tensor else None
    for alloc in nc.m.functions[0].allocations:
        if not isinstance(alloc, _mb.MemoryLocationSet):
            continue
        name = alloc.memorylocations[0].name
        if alloc.kind == "ExternalInput":
            if name != partition_name:
                in_names.append(name)
        elif alloc.kind == "ExternalOutput":
            out_names.append(name)
            shape = tuple(alloc.tensor_shape)
            dtype = _mb.dt.np(alloc.dtype)
            out_avals.append(jax.core.ShapedArray(shape, dtype))
            zero_outs.append(np.zeros(shape, dtype))
    n_params = len(in_names)
    all_in_names = list(in_names) + list(out_names)
    if partition_name is not None:
        all_in_names.append(partition_name)

    def _body(*args):
        ops = list(args)
        if partition_name is not None:
            ops = ops + [bass2jax.partition_id_tensor()]
        outs = _bass_exec_p.bind(
            *ops,
            out_avals=tuple(out_avals),
            in_names=tuple(all_in_names),
            out_names=tuple(out_names),
            lowering_input_output_aliases=(),
            sim_require_finite=True,
            sim_require_nnan=True,
            nc=nc,
        )
        return tuple(outs)

    devices = jax.devices()[:NCORES]
    mesh = Mesh(np.asarray(devices), ("core",))
    in_specs = (PartitionSpec("core"),) * (n_params + len(out_names))
    out_specs = (PartitionSpec("core"),) * len(out_names)
    f = jax.jit(shard_map(_body, mesh=mesh, in_specs=in_specs,
                          out_specs=out_specs, check_rep=False))
    concat_in = [np.concatenate([np.asarray(in_maps[c][n]) for c in range(NCORES)], axis=0)
                 for n in in_names]
    concat_zeros = [np.zeros((NCORES * z.shape[0], *z.shape[1:]), z.dtype) for z in zero_outs]
    args = [jax.device_put(a) for a in concat_in + concat_zeros]
    o = f(*args)
    jax.block_until_ready(o)
    times = []
    for _ in range(max(reps, 30)):
        t0 = time.perf_counter()
        o = f(*args)
        jax.block_until_ready(o)
        times.append(time.perf_counter() - t0)
    times.sort()
    return times[0] * 1e9, times


def bench_floor(reps=30):
    """Dispatch-floor: time a trivial 8-core NEFF (one 64KB copy)."""
    import time
    import jax
    from jax.sharding import Mesh, PartitionSpec
    from jax.experimental.shard_map import shard_map
    from concourse import bass2jax
    from concourse.bass2jax import _bass_exec_p
    import concourse.bacc as _bacc
    import concourse.tile as _tile

    if "floor_nc" not in _CACHE:
        nc = _bacc.Bacc(None, target_bir_lowering=False, debug=False)
        a = nc.dram_tensor("a", [128, 128], F32, kind="ExternalInput")
        b = nc.dram_tensor("b", [128, 128], F32, kind="ExternalOutput")
        with _tile.TileContext(nc) as tc:
            with tc.tile_pool(name="p", bufs=1) as p:
                t = p.tile([128, 128], F32)
                nc.sync.dma_start(t[:], a.ap())
                nc.sync.dma_start(b.ap(), t[:])
        nc.finalize()
        _CACHE["floor_nc"] = nc
    nc = _CACHE["floor_nc"]
    bass2jax.install_neuronx_cc_hook()
    partition_name = nc.partition_id_tensor.name if nc.partition_id_tensor else None
    in_names = ["a", "b"]
    if partition_name is not None:
        in_names.append(partition_name)
    out_avals = (jax.core.ShapedArray((128, 128), np.float32),)

    def _body(*args):
        ops = list(args)
        if partition_name is not None:
            ops = ops + [bass2jax.partition_id_tensor()]
        return tuple(_bass_exec_p.bind(
            *ops, out_avals=out_avals, in_names=tuple(in_names),
            out_names=("b",), lowering_input_output_aliases=(),
            sim_require_finite=True, sim_require_nnan=True, nc=nc))

    devices = jax.devices()[:NCORES]
    mesh = Mesh(np.asarray(devices), ("core",))
    f = jax.jit(shard_map(_body, mesh=mesh,
                          in_specs=(PartitionSpec("core"),) * 2,
                          out_specs=(PartitionSpec("core"),), check_rep=False))
    a = jax.device_put(np.zeros((NCORES * 128, 128), np.float32))
    z = jax.device_put(np.zeros((NCORES * 128, 128), np.float32))
    o = f(a, z); jax.block_until_ready(o)
    times = []
    for _ in range(reps):
        t0 = time.perf_counter()
        o = f(a, z)
        jax.block_until_ready(o)
        times.append(time.perf_counter() - t0)
    times.sort()
    return times[0] * 1e9


# revision 15
# speedup vs baseline: 4.0414x; 4.0414x over previous
"""Distributed Bass kernel for sparse cluster attention on 8 TRN2 NeuronCores.

v3: head-parallel compute, token-sharded inputs, minimal staged bytes.
  - Inputs per core (~6 MB): pre-tiled x^T token slice (bf16), keyframe-x
    hi slice, per-head W_qkv columns [q|k|v], W-lo for scores, W_proj row
    slice.  On-device AllGathers rebuild full x / keyframe-x / W_proj
    (~40 MB staged total vs 300 MB replicated).
  - Phase B: keyframe q.k scores for the core's 2 heads over all 2048
    keyframe tokens, 2-pass (W hi + W lo) for topk-exact selection,
    per-head reduce + max -> AllReduce(max) [2048] (hidden behind A).
  - Phase A: qkv for 2 heads over all 16384 tokens from gathered x:
    qT [128ch, N] direct; k/v channel-major then PE-transposed into
    token-major kv_sb [tok, k|v].
  - Phase 3: on-device top-153 per cluster via rank comparison.
  - Phase 3b: fused one-hot gathers: [sel, k|v] in one 256-wide matmul
    per token-chunk; k half transposed into kg.
  - Phase 4: flash-style attention per consumer cluster, software-
    pipelined: logits(g) [PE] -> exp(g) [ACT] -> AV(g-1) [PE].
  - Phase 5: output halves AllToAll'd while the other half computes;
    token-parallel proj -> out f16.
"""

import numpy as np
import ml_dtypes

import os
import concourse.bass as bass
import concourse.bacc as bacc
import concourse.mybir as mybir
import concourse.tile as tile
from concourse.tile_rust import add_dep_helper
from concourse.bass_utils import run_bass_kernel_spmd

BF16 = mybir.dt.bfloat16
F32 = mybir.dt.float32
F16 = mybir.dt.float16
I32 = mybir.dt.int32
AF = mybir.ActivationFunctionType
OP = mybir.AluOpType

# problem constants
H, D, C = 16, 64, 1024
S, P = 32, 512
K, FC = 4, 8
N = S * P                      # 16384 tokens
TK = 153                       # top-k patches per cluster
NSUB = 5                       # subsampled frames
NCORES = 8
HC = H // NCORES               # heads per core = 2
QCH = HC * D                   # q channels per core = 128
TOKS = N // NCORES             # tokens per core slice = 2048
KFT = K * P                    # keyframe tokens = 2048
KFC = KFT // NCORES            # keyframe tokens per core slice = 256
SCALE = float(D) ** -0.5
FULL = FC * TK                 # packed kv rows per full src block = 1224
PRE5 = NSUB * TK               # packed kv rows per 5-frame prefix = 765
KGW = 1280                     # kg tile width (>= FULL, mult of 128)
NT = N // 512                  # 32 token tiles of 512

_CACHE: dict = {}


def _chunks_for(ci):
    """(src, chunk, rows) list for consumer cluster ci over packed kv."""
    out = []
    for src in range(K):
        valid = FULL if src in (0, ci) else PRE5
        nch = (valid + 127) // 128
        for c in range(nch):
            out.append((src, c, min(128, valid - c * 128)))
    return out


def _win_frames(c):
    """Frames whose packed rows [f*153, (f+1)*153) intersect window
    [128c, 128(c+1))."""
    lo, hi = 128 * c, 128 * (c + 1)
    return [f for f in range(FC) if f * TK < hi and (f + 1) * TK > lo]


def build_nc(clusters, keyframes):
    STUB = os.environ.get("KSTUB", "0") == "1"
    KCUT = int(os.environ.get("KCUT", "9"))
    KREP = int(os.environ.get("KREP", "1"))
    KDBG = os.environ.get("KDBG", "0") == "1"
    nc = bacc.Bacc(None, target_bir_lowering=False, debug=False)

    # ---- kernel I/O (token-sharded, pre-tiled for contiguous DMA) ----
    # x^T slice as 4 SBUF-ready tiles: [q, p, a, t] = x[core*2048+q*512+t, a*128+p]
    xsT = nc.dram_tensor("xsT", [4, 128, 8, 512], BF16, kind="ExternalInput")
    # keyframe-x hi slice (kf_tok order), 2 tiles of 128 kf tokens
    xkfh = nc.dram_tensor("xkfh", [2, 128, 8, 128], BF16, kind="ExternalInput")
    # per-head weights, SBUF-ready [p, a, c]: [q 128 | k 128 | v 128]
    wqkv = nc.dram_tensor("wqkv", [128, 8, 3 * QCH], BF16, kind="ExternalInput")
    # W-lo for scores [p, a, c]: [q 128 | k 128]
    wqkl = nc.dram_tensor("wqkl", [128, 8, 2 * QCH], BF16, kind="ExternalInput")
    # W_proj row slice
    wpj_s = nc.dram_tensor("wpj_s", [128, C], BF16, kind="ExternalInput")
    bqs = nc.dram_tensor("bqs", [3 * QCH], F32, kind="ExternalInput")
    bpj = nc.dram_tensor("bpj", [C], F32, kind="ExternalInput")
    out_ext = nc.dram_tensor("out", [TOKS, C], F16, kind="ExternalOutput")
    if KDBG:
        dbg_qT = nc.dram_tensor("dbg_qT", [128, N], BF16, kind="ExternalOutput")
        dbg_kv = nc.dram_tensor("dbg_kv", [128, N // 128, 2 * QCH], BF16, kind="ExternalOutput")
        dbg_sc = nc.dram_tensor("dbg_sc", [KFT], F32, kind="ExternalOutput")
        dbg_kg = nc.dram_tensor("dbg_kg", [128, K * KGW], BF16, kind="ExternalOutput")
        dbg_va = nc.dram_tensor("dbg_va", [128, K * 10 * 130], BF16, kind="ExternalOutput")
        dbg_xk = nc.dram_tensor("dbg_xk", [16, 128, 8, 128], BF16, kind="ExternalOutput")
        dbg_qk = nc.dram_tensor("dbg_qk", [16, 128, 2 * QCH], F32, kind="ExternalOutput")

    # ---- internal DRAM ----
    xg_in = nc.dram_tensor("xg_in", [4, 128, 8, 512], BF16)
    xg = nc.dram_tensor("xg", [NCORES, 4, 128, 8, 512], BF16, addr_space="Shared")
    xk_in = nc.dram_tensor("xk_in", [2, 128, 8, 128], BF16)
    xk_g = nc.dram_tensor("xk_g", [NCORES, 2, 128, 8, 128], BF16, addr_space="Shared")
    wp_in = nc.dram_tensor("wp_in", [128, C], BF16)
    wp_g = nc.dram_tensor("wp_g", [NCORES, 128, C], BF16, addr_space="Shared")
    sc_in = nc.dram_tensor("sc_in", [KFT], F32)
    sc_out = nc.dram_tensor("sc_out", [KFT], F32, addr_space="Shared")
    ag_in = [nc.dram_tensor(f"ag_in{i}", [NCORES, QCH, TOKS // 2], BF16) for i in range(2)]
    ag_out = [nc.dram_tensor(f"ag_out{i}", [NCORES, QCH, TOKS // 2], BF16)
              for i in range(2)]

    def coll(kind, op, ins, outs):
        return nc.gpsimd.collective_compute(
            kind, op, replica_groups=[list(range(NCORES))], ins=ins, outs=outs)

    if STUB:
        with tile.TileContext(nc) as tc:
            with tc.tile_pool(name="sp", bufs=2) as sp:
                t = sp.tile([128, 512], BF16)
                nc.sync.dma_start(t[:], xsT.ap()[0, :, 0, :])
                t2 = sp.tile([128, 512], F16)
                nc.vector.tensor_copy(t2[:], t[:])
                nc.sync.dma_start(out_ext.ap()[0:128, 0:512], t2[:])
        nc.finalize()
        return nc

    # per-cluster frame lists and half split (A2A halves by token range)
    frames = [[int(clusters[ci][j]) for j in range(FC)] for ci in range(K)]
    half_qt = []
    for half in range(2):
        hq = {}
        for ci in range(K):
            hq[ci] = [j for j in range(FC)
                      if ((frames[ci][j] * P) % TOKS) // (TOKS // 2) == half]
        half_qt.append(hq)

    with tile.TileContext(nc) as tc:
        with (
            tc.tile_pool(name="persist", bufs=1) as pp,
            tc.tile_pool(name="work", bufs=2) as wp,
            tc.tile_pool(name="xp", bufs=2) as xp,
            tc.tile_pool(name="kp", bufs=3) as kp,
            tc.tile_pool(name="expw", bufs=3) as ep,
            tc.tile_pool(name="pslg", bufs=2, space="PSUM") as psL,
            tc.tile_pool(name="psav", bufs=2, space="PSUM") as psV,
            tc.tile_pool(name="psmed", bufs=2, space="PSUM") as psM,
        ):
            # ============ gather collectives first (x, kf-x, Wproj) ========
            nc.sync.dma_start(xk_in.ap(), xkfh.ap())
            xk_coll = coll("AllGather", OP.bypass, [xk_in.ap().opt()], [xk_g.ap().opt()])
            nc.sync.dma_start(xg_in.ap(), xsT.ap())
            xg_coll = coll("AllGather", OP.bypass, [xg_in.ap().opt()], [xg.ap().opt()])
            nc.sync.dma_start(wp_in.ap(), wpj_s.ap())
            wp_coll = coll("AllGather", OP.bypass, [wp_in.ap().opt()], [wp_g.ap().opt()])

            # ================= persistent SBUF + weights =================
            qT = pp.tile([128, N], BF16, tag="qT")                  # 32KB/p
            kv_sb = pp.tile([128, N // 128, 2 * QCH], BF16, tag="kvsb")  # 64KB/p
            kg = pp.tile([128, K, KGW], BF16, tag="kg")             # 10KB/p
            vaug = pp.tile([128, K, 10, 130], BF16, tag="vaug")     # 10.2KB/p

            wqkv_sb = pp.tile([128, 8, 3 * QCH], BF16, tag="wqkv")
            nc.sync.dma_start(wqkv_sb[:], wqkv.ap())
            wqkl_sb = pp.tile([128, 8, 2 * QCH], BF16, tag="wqkl")
            nc.sync.dma_start(wqkl_sb[:], wqkl.ap())

            # biases: bqs = [q 128 | k 128 | v 128] f32 -> bf16 row
            bq_f = wp.tile([1, 3 * QCH], F32, tag="srow", bufs=1, name="bq_f")
            nc.sync.dma_start(bq_f[:], bqs.ap().rearrange("(a c) -> a c", a=1))
            bq_b = pp.tile([1, 3 * QCH], BF16, tag="bqb")
            nc.vector.tensor_copy(bq_b[:], bq_f[:])
            bpj_f = wp.tile([1, C], F32, tag="srow", bufs=1, name="bpj_f")
            nc.sync.dma_start(bpj_f[:], bpj.ap().rearrange("(a c) -> a c", a=1))
            bpj_b = pp.tile([1, C], BF16, tag="bpjb")
            nc.vector.tensor_copy(bpj_b[:], bpj_f[:])

            ones_b = pp.tile([1, 512], BF16, tag="onesb")
            nc.vector.memset(ones_b[:], 1.0)
            onesf = pp.tile([1, 128], F32, tag="onesf")
            nc.vector.memset(onesf[:], 1.0)

            # identity matrix + iotas
            it_row = wp.tile([128, 128], I32, tag="i32w", bufs=1)
            nc.gpsimd.iota(it_row[:], pattern=[[1, 128]], base=0, channel_multiplier=0)
            it_col = wp.tile([128, 1], I32, tag="i32c", bufs=1)
            nc.gpsimd.iota(it_col[:], pattern=[[1, 1]], base=0, channel_multiplier=1)
            rowf = wp.tile([128, 128], F32, tag="rowf", bufs=1)
            nc.vector.tensor_copy(rowf[:], it_row[:])
            colf = wp.tile([128, 1], F32, tag="colf", bufs=1)
            nc.vector.tensor_copy(colf[:], it_col[:])
            ident = pp.tile([128, 128], BF16, tag="ident")
            nc.vector.tensor_scalar(ident[:], rowf[:], colf[:], None, OP.is_equal)

            iota_tc = wp.tile([128, 128], I32, tag="i32w", bufs=1, name="iota_tc")
            nc.gpsimd.iota(iota_tc[:], pattern=[[128, 128]], base=0, channel_multiplier=1)
            iota_tcf = pp.tile([128, 128], F32, tag="iotcf")
            nc.vector.tensor_copy(iota_tcf[:], iota_tc[:])
            iota160 = wp.tile([128, 160], I32, tag="i32w", bufs=1, name="iota160")
            nc.gpsimd.iota(iota160[:], pattern=[[1, 160]], base=0, channel_multiplier=0)
            iota160f = pp.tile([128, 160], F32, tag="io160f")
            nc.vector.tensor_copy(iota160f[:], iota160[:])
            iota_pv = wp.tile([128, 4], I32, tag="i32c", bufs=1, name="iota_pv")
            nc.gpsimd.iota(iota_pv[:], pattern=[[128, 4]], base=0, channel_multiplier=1)
            iota_pvf = pp.tile([128, 4], F32, tag="iopvf")
            nc.vector.tensor_copy(iota_pvf[:], iota_pv[:])

            wpj_sb = pp.tile([128, 8, C], BF16, tag="wpj")
            _i = nc.sync.dma_start(wpj_sb[:], wp_g.ap().rearrange("a p c -> p a c"))
            add_dep_helper(_i.ins, wp_coll.ins, reason="wpj read after AG")

            for rep in range(KREP):
              # ================= phase B: keyframe scores =================
              if KCUT >= 1:
                for t2 in range(KFT // 128):
                    xkh = kp.tile([128, 8, 128], BF16, tag="xkf", name=f"xkh{rep}_{t2}")
                    _i = nc.sync.dma_start(xkh[:], xk_g.ap()[t2 // 2, t2 % 2])
                    add_dep_helper(_i.ins, xk_coll.ins, reason="xkh read after AG")
                    if KDBG:
                        nc.sync.dma_start(dbg_xk.ap()[t2], xkh[:])
                    psb = psL.tile([128, 2 * QCH], F32, tag="lg", name=f"psb{rep}_{t2}")
                    for cc in range(8):
                        # pass 1: W-hi ([q|k] adjacent in wqkv cols 0:256)
                        nc.tensor.matmul(psb[:], xkh[:, cc, :],
                                         wqkv_sb[:, cc, 0:2 * QCH],
                                         start=(cc == 0), stop=False)
                    for cc in range(8):
                        # pass 2: W-lo
                        nc.tensor.matmul(psb[:], xkh[:, cc, :], wqkl_sb[:, cc, :],
                                         start=False, stop=False)
                    nc.tensor.matmul(psb[:], ones_b[:, 0:128], bq_b[:, 0:2 * QCH],
                                     start=False, stop=True)
                    qk_s = wp.tile([128, 2 * QCH], F32, tag="qks", bufs=1)
                    nc.vector.tensor_copy(qk_s[:], psb[:])
                    if KDBG:
                        nc.sync.dma_start(dbg_qk.ap()[t2], qk_s[:])
                    qs = wp.tile([128, QCH], F32, tag="qs", bufs=1)
                    nc.vector.tensor_tensor(qs[:], qk_s[:, 0:QCH], qk_s[:, QCH:2 * QCH],
                                            OP.mult)
                    hs = wp.tile([128, HC], F32, tag="hs", bufs=2)
                    for h in range(HC):
                        nc.vector.reduce_sum(hs[:, h:h + 1], qs[:, h * D:(h + 1) * D],
                                             axis=mybir.AxisListType.X)
                    smax_c = wp.tile([128, 1], F32, tag="smaxc", bufs=2)
                    nc.vector.reduce_max(smax_c[:], hs[:], axis=mybir.AxisListType.X)
                    nc.sync.dma_start(
                        sc_in.ap()[t2 * 128:(t2 + 1) * 128].rearrange("(p a) -> p a", a=1),
                        smax_c[:])
                sc_coll = coll("AllReduce", OP.max, [sc_in.ap().opt()], [sc_out.ap().opt()])

              # ================= phase A: qkv all tokens, 2 heads ===========
              if KCUT >= 2:
                for tt in range(NT):
                    xt = xp.tile([128, 8, 512], BF16, tag="xt", name=f"xt{rep}_{tt}")
                    _i = nc.sync.dma_start(xt[:], xg.ap()[tt // 4, tt % 4])
                    add_dep_helper(_i.ins, xg_coll.ins, reason="xt read after AG")
                    psq = psL.tile([128, 512], F32, tag="lg", name=f"psq{rep}_{tt}")
                    for cc in range(8):
                        nc.tensor.matmul(psq[:], wqkv_sb[:, cc, 0:QCH], xt[:, cc, :],
                                         start=(cc == 0), stop=False)
                    nc.tensor.matmul(psq[:], bq_b[:, 0:QCH], ones_b[:],
                                     start=False, stop=True)
                    nc.scalar.copy(qT[:, tt * 512:(tt + 1) * 512], psq[:])
                    for kvc in range(2):
                        pskv = psL.tile([128, 512], F32, tag="lg",
                                        name=f"pskv{rep}_{tt}_{kvc}")
                        for cc in range(8):
                            nc.tensor.matmul(pskv[:],
                                             wqkv_sb[:, cc, (1 + kvc) * 128:(2 + kvc) * 128],
                                             xt[:, cc, :], start=(cc == 0), stop=False)
                        nc.tensor.matmul(pskv[:], bq_b[:, (1 + kvc) * 128:(2 + kvc) * 128],
                                         ones_b[:], start=False, stop=True)
                        kvT = ep.tile([128, 512], BF16, tag="kvT", bufs=2)
                        nc.vector.tensor_copy(kvT[:], pskv[:])
                        pst = psM.tile([128, 512], BF16, tag="med",
                                       name=f"pst{rep}_{tt}_{kvc}")
                        for s4 in range(4):
                            nc.tensor.transpose(pst[:, s4 * 128:(s4 + 1) * 128],
                                                kvT[:, s4 * 128:(s4 + 1) * 128], ident[:])
                        nc.vector.tensor_copy(
                            kv_sb[:, tt * 4:(tt + 1) * 4, kvc * 128:(kvc + 1) * 128],
                            pst[:].rearrange("p (a k) -> p a k", k=128))

              if KDBG and KCUT >= 2:
                nc.sync.dma_start(dbg_qT.ap(), qT[:])
                nc.sync.dma_start(dbg_kv.ap(), kv_sb[:])
              if KDBG and KCUT >= 1:
                scdbg = wp.tile([128, 16], F32, tag="qks", bufs=1, name="scdbg")
                _i = nc.sync.dma_start(scdbg[:], sc_out.ap().rearrange("(t p) -> p t", p=128))
                add_dep_helper(_i.ins, sc_coll.ins, reason="scdbg read after AR")
                nc.sync.dma_start(dbg_sc.ap().rearrange("(t p) -> p t", p=128), scdbg[:])
              # ================= phase 3: top-k -> packed patch-id rows =====
              psel_rows = {}
              if KCUT >= 3:
                for cl in range(K):
                    s_row = wp.tile([1, P], F32, tag="srow", bufs=1)
                    _i = nc.sync.dma_start(
                        s_row[:], sc_out.ap()[cl * P:(cl + 1) * P]
                        .rearrange("(a c) -> a c", a=1))
                    add_dep_helper(_i.ins, sc_coll.ins, reason="s_row read after AR")
                    s_colT = wp.tile([128, 4], F32, tag="scolT", bufs=1)
                    _i = nc.sync.dma_start(
                        s_colT[:], sc_out.ap()[cl * P:(cl + 1) * P]
                        .rearrange("(a p) -> p a", p=128))
                    add_dep_helper(_i.ins, sc_coll.ins, reason="s_colT read after AR")
                    ps_bc = psM.tile([128, P], F32, tag="med", name=f"psbc{rep}_{cl}")
                    nc.tensor.matmul(ps_bc[:], onesf[:], s_row[:], start=True, stop=True)
                    s_bc = wp.tile([128, P], F32, tag="sbc", bufs=1)
                    nc.vector.tensor_copy(s_bc[:], ps_bc[:])
                    ps_row = psM.tile([1, 160], F32, tag="med", name=f"psrow{rep}_{cl}")
                    for pc in range(4):
                        gt = wp.tile([128, P], BF16, tag="gtm", bufs=1)
                        nc.vector.tensor_scalar(gt[:], s_bc[:], s_colT[:, pc:pc + 1],
                                                None, OP.is_gt)
                        rank = wp.tile([128, 1], F32, tag="rank", bufs=2)
                        nc.vector.reduce_sum(rank[:], gt[:], axis=mybir.AxisListType.X)
                        eqr = wp.tile([128, 160], F32, tag="eqr", bufs=1)
                        nc.vector.tensor_scalar(eqr[:], iota160f[:], rank[:],
                                                None, OP.is_equal)
                        nc.tensor.matmul(ps_row[:], iota_pvf[:, pc:pc + 1], eqr[:],
                                         start=(pc == 0), stop=(pc == 3))
                    psel_row = pp.tile([1, 160], F32, tag=f"pselr{cl}",
                                       name=f"pselr{rep}_{cl}")
                    nc.vector.tensor_copy(psel_row[:], ps_row[:])
                    psel_rows[cl] = psel_row

                # ---- phase 3b: fused one-hot gathers ----
                for src in range(K):
                    psB2 = wp.tile([128, KGW], F32, tag="psB2", bufs=1)
                    nc.vector.memset(psB2[:, FULL:KGW], -1.0)
                    for f8 in range(FC):
                        fr = frames[src][f8]
                        ps_b = psM.tile([128, 512], F32, tag="med",
                                        name=f"psb2_{rep}_{src}_{f8}")
                        nc.tensor.matmul(ps_b[:, 0:160], onesf[:], psel_rows[src][:],
                                         start=True, stop=True)
                        nc.vector.tensor_scalar(psB2[:, f8 * TK:(f8 + 1) * TK],
                                                ps_b[:, 0:TK], float(fr * P), None, OP.add)
                    for c in range(10):
                        tcs = []
                        for f8 in _win_frames(c):
                            fr = frames[src][f8]
                            tcs.extend(fr * 4 + i for i in range(4))
                        psg = psM.tile([128, 2 * QCH], F32, tag="med",
                                       name=f"psg{rep}_{src}_{c}")
                        for ti, tc_ in enumerate(tcs):
                            ohW = wp.tile([128, 128], BF16, tag="ohW", bufs=2,
                                          name=f"ohW{rep}_{src}_{c}_{ti}")
                            nc.vector.tensor_scalar(ohW[:], psB2[:, c * 128:(c + 1) * 128],
                                                    iota_tcf[:, tc_:tc_ + 1], None,
                                                    OP.is_equal)
                            nc.tensor.matmul(psg[:], ohW[:], kv_sb[:, tc_, :],
                                             start=(ti == 0), stop=(ti == len(tcs) - 1))
                        stage = wp.tile([128, 2 * QCH], BF16, tag="stg", bufs=2)
                        nc.vector.tensor_copy(stage[:], psg[:])
                        nc.vector.tensor_copy(vaug[:, src, c, 0:64], stage[:, 128:192])
                        nc.vector.tensor_copy(vaug[:, src, c, 65:129], stage[:, 192:256])
                        pst2 = psM.tile([128, 128], BF16, tag="med",
                                        name=f"pst2_{rep}_{src}_{c}")
                        nc.tensor.transpose(pst2[:], stage[:, 0:128], ident[:])
                        nc.vector.tensor_copy(kg[:, src, c * 128:(c + 1) * 128], pst2[:])
                nc.vector.memset(vaug[:, :, :, 64:65], 1.0)
                nc.vector.memset(vaug[:, :, :, 129:130], 1.0)

              if KDBG and KCUT >= 3:
                nc.sync.dma_start(dbg_kg.ap(), kg[:].rearrange("p a b -> p (a b)"))
                nc.sync.dma_start(dbg_va.ap(), vaug[:].rearrange("p a b c -> p (a b c)"))
              # ================= phase 4: attention, half-pipelined =========
              a2a_colls = {}
              if KCUT >= 4:
                for half in range(2):
                    for ci in range(K):
                        chunks = _chunks_for(ci)
                        for qt in half_qt[half][ci]:
                            f_q = frames[ci][qt]
                            qsl = slice(f_q * P, (f_q + 1) * P)
                            ps_av = [psV.tile([65, 512], F32, tag="av",
                                              name=f"psav{rep}_{ci}_{qt}_{i}")
                                     for i in range(2)]
                            pend = None  # (g, ew, rows, src, c) awaiting AV
                            for g, (src, c, rows) in enumerate(chunks):
                                ps_lg = psL.tile([128, 1024], F32, tag="lg", bufs=2,
                                                 name=f"pslg{rep}_{ci}_{qt}_{g}")
                                for h in range(2):
                                    nc.tensor.matmul(
                                        ps_lg[:, h * 512:(h + 1) * 512],
                                        kg[h * 64:(h + 1) * 64, src, c * 128:(c + 1) * 128],
                                        qT[h * 64:(h + 1) * 64, qsl],
                                        start=True, stop=True,
                                        tile_position=(h * 64, 0))
                                ew = ep.tile([128, 1024], BF16, tag="ew", bufs=2)
                                nc.scalar.activation(ew[:], ps_lg[:], AF.Exp, scale=SCALE)
                                if pend is not None:
                                    pg, pew, prows, psrc, pc_ = pend
                                    for h in range(2):
                                        nc.tensor.matmul(
                                            ps_av[h][:],
                                            vaug[0:prows, psrc, pc_, h * 65:(h + 1) * 65],
                                            pew[0:prows, h * 512:(h + 1) * 512],
                                            start=(pg == 0), stop=False)
                                pend = (g, ew, rows, src, c)
                            pg, pew, prows, psrc, pc_ = pend
                            for h in range(2):
                                nc.tensor.matmul(
                                    ps_av[h][:],
                                    vaug[0:prows, psrc, pc_, h * 65:(h + 1) * 65],
                                    pew[0:prows, h * 512:(h + 1) * 512],
                                    start=(pg == 0), stop=True)
                            # normalize -> A2A staging
                            otile = ep.tile([128, 512], BF16, tag="ot", bufs=2)
                            for h in range(2):
                                rec = wp.tile([1, 512], F32, tag="rec", bufs=1)
                                nc.vector.reciprocal(rec[:], ps_av[h][64:65, :])
                                ps_bc2 = psM.tile([64, 512], F32, tag="med",
                                                  name=f"psbc2_{rep}_{ci}_{qt}_{h}")
                                nc.tensor.matmul(ps_bc2[:], onesf[:, 0:64], rec[:],
                                                 start=True, stop=True)
                                bc_sb = wp.tile([64, 512], F32, tag="bcsb", bufs=1)
                                nc.vector.tensor_copy(bc_sb[:], ps_bc2[:])
                                nc.vector.tensor_tensor(
                                    otile[h * 64:(h + 1) * 64, :],
                                    ps_av[h][0:64, :], bc_sb[:], OP.mult)
                            jcore = (f_q * P) // TOKS
                            toff = (f_q * P) % TOKS % (TOKS // 2)
                            nc.sync.dma_start(
                                ag_in[half].ap()[jcore, :, toff:toff + 512], otile[:])
                    a2a_colls[half] = coll(
                        "AllToAll", OP.bypass,
                        [ag_in[half].ap().opt()], [ag_out[half].ap().opt()])

              # ================= phase 5: proj, half-pipelined ==============
              if KCUT >= 5:
                for half in range(2):
                    for quarter in range(2):
                        atk = xp.tile([128, 8, 512], BF16, tag="xt",
                                      name=f"atk{rep}_{half}_{quarter}")
                        _i = nc.sync.dma_start(
                            atk[:],
                            ag_out[half].ap()[:, :, quarter * 512:(quarter + 1) * 512]
                            .rearrange("j p t -> p j t"))
                        add_dep_helper(_i.ins, a2a_colls[half].ins,
                                       reason="atk read after A2A")
                        for mt in range(4):
                            gmt = half * 8 + quarter * 4 + mt
                            for ntile in range(2):
                                nsl = slice(ntile * 512, (ntile + 1) * 512)
                                ps = psM.tile([128, 512], F32, tag="med",
                                              name=f"pspj{rep}_{gmt}_{ntile}")
                                for cc in range(8):
                                    nc.tensor.matmul(ps[:], atk[:, cc, mt * 128:(mt + 1) * 128],
                                                     wpj_sb[:, cc, nsl],
                                                     start=(cc == 0), stop=False)
                                nc.tensor.matmul(ps[:], ones_b[:, 0:128], bpj_b[:, nsl],
                                                 start=False, stop=True)
                                ot = wp.tile([128, 512], F16, tag="otile", bufs=2)
                                nc.scalar.copy(ot[:], ps[:])
                                nc.sync.dma_start(
                                    out_ext.ap()[gmt * 128:(gmt + 1) * 128, nsl], ot[:])

    nc.finalize()
    return nc


def _host_prep(x, W_qkv, b_qkv, W_proj, b_proj, clusters, keyframes):
    bf = ml_dtypes.bfloat16
    x2 = np.ascontiguousarray(x.reshape(N, C))
    kf_tok = np.concatenate([np.arange(P, dtype=np.int64) + int(f) * P
                             for f in keyframes])
    xkf_h = x2[kf_tok].astype(bf)                                  # [2048, C]
    bpj_f = np.ascontiguousarray(b_proj.astype(np.float32))

    def tile_xT(xs_bf16, ntile, tw):
        # [tok, C] -> [ntile, 128, 8, tw]: [q, p, a, t] = x[q*tw+t, a*128+p]
        return np.ascontiguousarray(
            xs_bf16.reshape(ntile, tw, 8, 128).transpose(0, 3, 2, 1))

    in_maps = []
    for core in range(NCORES):
        h0 = core * HC
        qcols = np.arange(h0 * D, h0 * D + QCH)
        wq_s = W_qkv[:, qcols]
        wk_s = W_qkv[:, C + qcols]
        wv_s = W_qkv[:, 2 * C + qcols]
        wqkv_f = np.concatenate([wq_s, wk_s, wv_s], axis=1)        # [C, 384] f32
        wqkv_b = wqkv_f.astype(bf)
        wqk_lo = (wqkv_f[:, 0:2 * QCH]
                  - wqkv_b[:, 0:2 * QCH].astype(np.float32)).astype(bf)
        bqs_s = np.concatenate([b_qkv[qcols], b_qkv[C + qcols],
                                b_qkv[2 * C + qcols]]).astype(np.float32)
        xs = x2[core * TOKS:(core + 1) * TOKS].astype(bf)          # [2048, C]
        xkf_s = xkf_h[core * KFC:(core + 1) * KFC]                 # [256, C]
        in_maps.append({
            "xsT": tile_xT(xs, 4, 512),
            "xkfh": tile_xT(xkf_s, 2, 128),
            "wqkv": np.ascontiguousarray(
                wqkv_b.reshape(8, 128, 3 * QCH).transpose(1, 0, 2)),
            "wqkl": np.ascontiguousarray(
                wqk_lo.reshape(8, 128, 2 * QCH).transpose(1, 0, 2)),
            "wpj_s": np.ascontiguousarray(
                W_proj[core * 128:(core + 1) * 128].astype(bf)),
            "bqs": np.ascontiguousarray(bqs_s),
            "bpj": bpj_f,
        })
    return in_maps


def kernel(x, W_qkv, b_qkv, W_proj, b_proj, clusters, keyframes, **run_kwargs):
    x = np.asarray(x, dtype=np.float32)
    W_qkv = np.asarray(W_qkv, dtype=np.float32)
    b_qkv = np.asarray(b_qkv, dtype=np.float32)
    W_proj = np.asarray(W_proj, dtype=np.float32)
    b_proj = np.asarray(b_proj, dtype=np.float32)
    clusters = np.asarray(clusters, dtype=np.int32)
    keyframes = np.asarray(keyframes, dtype=np.int32)

    key = (clusters.tobytes(), keyframes.tobytes(), os.environ.get("KSTUB"),
           os.environ.get("KCUT"), os.environ.get("KREP"), os.environ.get("KDBG"))
    if _CACHE.get("key") != key:
        _CACHE["nc"] = build_nc(clusters, keyframes)
        _CACHE["key"] = key
    nc = _CACHE["nc"]

    in_maps = _host_prep(x, W_qkv, b_qkv, W_proj, b_proj, clusters, keyframes)
    res = run_bass_kernel_spmd(nc, in_maps, core_ids=list(range(NCORES)), **run_kwargs)
    _CACHE["last_result"] = res
    outs = res.results
    full = np.concatenate([np.asarray(outs[c]["out"], dtype=np.float32)
                           for c in range(NCORES)], axis=0)
    return full.reshape(1, N, C)


def bench(x, W_qkv, b_qkv, W_proj, b_proj, clusters, keyframes, iters=10, reps=5):
    """Steady-state on-device timing: times the best of `reps` calls."""
    import time
    import jax
    from jax.sharding import Mesh, PartitionSpec
    from jax.experimental.shard_map import shard_map
    from concourse import bass2jax
    from concourse.bass2jax import _bass_exec_p
    import concourse.mybir as _mb

    clusters = np.asarray(clusters, dtype=np.int32)
    keyframes = np.asarray(keyframes, dtype=np.int32)
    key = (clusters.tobytes(), keyframes.tobytes(), os.environ.get("KSTUB"),
           os.environ.get("KCUT"), os.environ.get("KREP"), os.environ.get("KDBG"))
    if _CACHE.get("key") != key:
        _CACHE["nc"] = build_nc(clusters, keyframes)
        _CACHE["key"] = key
    nc = _CACHE["nc"]
    bass2jax.install_neuronx_cc_hook()

    in_maps = _host_prep(np.asarray(x, np.float32), np.asarray(W_qkv, np.float32),
                         np.asarray(b_qkv, np.float32), np.asarray(W_proj, np.float32),
                         np.asarray(b_proj, np.float32), clusters, keyframes)

    in_names, out_names, out_avals, zero_outs = [], [], [], []
    partition_name = nc.partition_id_tensor.name if nc.partition_id_tensor else None
    for alloc in nc.m.functions[0].allocations:
        if not isinstance(alloc, _mb.MemoryLocationSet):
            continue
        name = alloc.memorylocations[0].name
        if alloc.kind == "ExternalInput":
            if name != partition_name:
                in_names.append(name)
        elif alloc.kind == "ExternalOutput":
            out_names.append(name)
            shape = tuple(alloc.tensor_shape)
            dtype = _mb.dt.np(alloc.dtype)
            out_avals.append(jax.core.ShapedArray(shape, dtype))
            zero_outs.append(np.zeros(shape, dtype))
    n_params = len(in_names)
    all_in_names = list(in_names) + list(out_names)
    if partition_name is not None:
        all_in_names.append(partition_name)

    def _body(*args):
        ops = list(args)
        if partition_name is not None:
            ops = ops + [bass2jax.partition_id_tensor()]
        outs = _bass_exec_p.bind(
            *ops,
            out_avals=tuple(out_avals),
            in_names=tuple(all_in_names),
            out_names=tuple(out_names),
            lowering_input_output_aliases=(),
            sim_require_finite=True,
            sim_require_nnan=True,
            nc=nc,
        )
        return tuple(outs)

    devices = jax.devices()[:NCORES]
    mesh = Mesh(np.asarray(devices), ("core",))
    in_specs = (PartitionSpec("core"),) * (n_params + len(out_names))
    out_specs = (PartitionSpec("core"),) * len(out_names)
    f = jax.jit(shard_map(_body, mesh=mesh, in_specs=in_specs,
                          out_specs=out_specs, check_rep=False))
    concat_in = [np.concatenate([np.asarray(in_maps[c][n]) for c in range(NCORES)], axis=0)
                 for n in in_names]
    concat_zeros = [np.zeros((NCORES * z.shape[0], *z.shape[1:]), z.dtype) for z in zero_outs]
    args = [jax.device_put(a) for a in concat_in + concat_zeros]
    o = f(*args)
    jax.block_until_ready(o)
    times = []
    for _ in range(max(reps, 30)):
        t0 = time.perf_counter()
        o = f(*args)
        jax.block_until_ready(o)
        times.append(time.perf_counter() - t0)
    times.sort()
    return times[0] * 1e9, times


def bench_floor(reps=30):
    """Dispatch-floor: time a trivial 8-core NEFF (one 64KB copy)."""
    import time
    import jax
    from jax.sharding import Mesh, PartitionSpec
    from jax.experimental.shard_map import shard_map
    from concourse import bass2jax
    from concourse.bass2jax import _bass_exec_p
    import concourse.bacc as _bacc
    import concourse.tile as _tile

    if "floor_nc" not in _CACHE:
        nc = _bacc.Bacc(None, target_bir_lowering=False, debug=False)
        a = nc.dram_tensor("a", [128, 128], F32, kind="ExternalInput")
        b = nc.dram_tensor("b", [128, 128], F32, kind="ExternalOutput")
        with _tile.TileContext(nc) as tc:
            with tc.tile_pool(name="p", bufs=1) as p:
                t = p.tile([128, 128], F32)
                nc.sync.dma_start(t[:], a.ap())
                nc.sync.dma_start(b.ap(), t[:])
        nc.finalize()
        _CACHE["floor_nc"] = nc
    nc = _CACHE["floor_nc"]
    bass2jax.install_neuronx_cc_hook()
    partition_name = nc.partition_id_tensor.name if nc.partition_id_tensor else None
    in_names = ["a", "b"]
    if partition_name is not None:
        in_names.append(partition_name)
    out_avals = (jax.core.ShapedArray((128, 128), np.float32),)

    def _body(*args):
        ops = list(args)
        if partition_name is not None:
            ops = ops + [bass2jax.partition_id_tensor()]
        return tuple(_bass_exec_p.bind(
            *ops, out_avals=out_avals, in_names=tuple(in_names),
            out_names=("b",), lowering_input_output_aliases=(),
            sim_require_finite=True, sim_require_nnan=True, nc=nc))

    devices = jax.devices()[:NCORES]
    mesh = Mesh(np.asarray(devices), ("core",))
    f = jax.jit(shard_map(_body, mesh=mesh,
                          in_specs=(PartitionSpec("core"),) * 2,
                          out_specs=(PartitionSpec("core"),), check_rep=False))
    a = jax.device_put(np.zeros((NCORES * 128, 128), np.float32))
    z = jax.device_put(np.zeros((NCORES * 128, 128), np.float32))
    o = f(a, z); jax.block_until_ready(o)
    times = []
    for _ in range(reps):
        t0 = time.perf_counter()
        o = f(a, z)
        jax.block_until_ready(o)
        times.append(time.perf_counter() - t0)
    times.sort()
    return times[0] * 1e9
